# revision 1
# baseline (speedup 1.0000x reference)
"""Conv1d (B=64, C_in=300, L=2048 -> C_out=512, K=3, pad=1) on 8 trn2 cores.

Strategy: data-parallel over batch (8 batches per core). Per batch, the
conv is computed as 8 accumulating float32r (TF32-mode, full-rate)
matmuls per (co_chunk, l_chunk) PSUM tile. The contraction (ci, k) of
900 rows is packed into 8 stationary chunks:

  c0/c1: k=0, ci 0-127 / 128-255   -> padded x   at window offset l0
  c2/c3: k=1, ci 0-127 / 128-255   -> unpadded x at window offset l0
  c4/c5: k=2, ci 0-127 / 128-255   -> padded x   at window offset l0+2
  c6:    k=2, ci 256-299 (44 rows) -> padded x   at window offset l0+2
  c7:    k=0 ci 256-299 + k=1 ci 256-299 merged (88 rows) -> m_sb at l0

The k=1 taps read a separate unpadded-x SBUF copy so every moving-operand
window starts at an even element offset (f32r streams slower from odd 4B
offsets), and the merged c7 chunk reads a small materialized tile whose
first 44 partitions hold padded-x rows ci 256-299 and next 44 partitions
hold unpadded-x rows ci 256-299. Host pre-packs the matching stationary
weight chunks. x is zero-padded to length 2050 on the host (f32r matmuls
require even column counts and 8B-aligned PSUM offsets, so SBUF-side
boundary trims are not legal). Weights stay stationary across the 4
l-chunks of each accumulation pass; bias is folded in during PSUM
evacuation on the vector engine.
"""

import contextlib

import numpy as np

import concourse.bass as bass
import concourse.mybir as mybir
import concourse.tile as tile
from concourse import bacc
from concourse.bass_utils import run_bass_kernel_spmd

B, C_IN, L = 64, 300, 2048
C_OUT, K = 512, 3
N_CORES = 8
B_LOC = B // N_CORES
LP = L + 2  # host-side zero-padded length
N_COC = C_OUT // 128  # co chunks of 128 partitions
LC = 512  # l chunk = one PSUM bank of fp32
N_LC = L // LC

# (rows, source, cic_or_None, window_offset) per stationary chunk;
# sources: 0 = x_sb (padded), 1 = x1_sb (unpadded), 2 = m_sb (merged tail)
CHUNKS = [
    (128, 0, 0, 0),
    (128, 0, 1, 0),
    (128, 1, 0, 0),
    (128, 1, 1, 0),
    (128, 0, 0, 2),
    (128, 0, 1, 2),
    (44, 0, 2, 2),
    (88, 2, None, 0),
]
N_CHUNK = len(CHUNKS)

_NC_CACHE = {}


def _build_nc(reps=1, probe=()):
    f32 = mybir.dt.float32
    f32r = mybir.dt.float32r
    nc = bacc.Bacc(None, target_bir_lowering=False)

    x_d = nc.dram_tensor("x", [B_LOC, C_IN, LP], f32r, kind="ExternalInput")
    w_d = nc.dram_tensor("w", [N_CHUNK, 128, C_OUT], f32r, kind="ExternalInput")
    b_d = nc.dram_tensor("b", [N_COC, 128, 1], f32, kind="ExternalInput")
    o_d = nc.dram_tensor("out", [B_LOC, C_OUT, L], f32, kind="ExternalOutput")

    with tile.TileContext(nc) as tc:
        with (
            tc.tile_pool(name="wpool", bufs=1) as wpool,
            tc.tile_pool(name="xpool", bufs=2) as xpool,
            tc.tile_pool(name="opool", bufs=3) as opool,
            tc.tile_pool(name="pspool", bufs=8, space="PSUM") as pspool,
        ):
            w_sb = wpool.tile([128, N_CHUNK, C_OUT], f32r)
            for c, (rc, _, _, _) in enumerate(CHUNKS):
                nc.sync.dma_start(out=w_sb[0:rc, c, :], in_=w_d[c, 0:rc, :])
            bias_sb = wpool.tile([128, N_COC], f32)
            for coc in range(N_COC):
                nc.sync.dma_start(out=bias_sb[:, coc : coc + 1], in_=b_d[coc])

            if reps > 1:
                # Benchmark mode: repeat the whole body inside the NEFF so
                # per-iteration HW time can be isolated from RPC/transfer
                # overhead by differencing two rep counts.
                rep_stack = contextlib.ExitStack()
                rep_stack.enter_context(
                    tc.For_i(
                        0,
                        reps,
                        1,
                        hint_engines=(
                            mybir.EngineType.PE,
                            mybir.EngineType.DVE,
                            mybir.EngineType.SP,
                        ),
                    )
                )
            else:
                rep_stack = contextlib.ExitStack()

            with rep_stack:
                for b in range(B_LOC):
                    x_sb = xpool.tile([128, 3, LP], f32r, name="x_sb", tag="x")
                    x1_sb = xpool.tile(
                        [128, 2, L], f32r, name="x1_sb", tag="x1"
                    )
                    m_sb = xpool.tile([128, LP], f32r, name="m_sb", tag="m")
                    srcs = (x_sb, x1_sb, m_sb)
                    for cic, c0 in ((0, 0), (1, 128), (2, 256)):
                        cs = min(128, C_IN - c0)
                        nc.sync.dma_start(
                            out=x_sb[0:cs, cic, :], in_=x_d[b, c0 : c0 + cs, :]
                        )
                    for cic, c0 in ((0, 0), (1, 128)):
                        nc.sync.dma_start(
                            out=x1_sb[0:128, cic, :],
                            in_=x_d[b, c0 : c0 + 128, 1 : L + 1],
                        )
                    nc.sync.dma_start(
                        out=m_sb[0:44, :], in_=x_d[b, 256:300, :]
                    )
                    nc.sync.dma_start(
                        out=m_sb[44:88, 0:L], in_=x_d[b, 256:300, 1 : L + 1]
                    )

                    for coc in range(N_COC):
                        psums = [
                            pspool.tile([128, LC], f32, name="ps", tag="ps")
                            for _ in range(N_LC)
                        ]
                        # Weight-stationary: all 4 l-chunks per chunk.
                        for c, (rc, src, cic, woff) in enumerate(CHUNKS):
                            lhsT = w_sb[0:rc, c, coc * 128 : (coc + 1) * 128]
                            for lc in range(N_LC):
                                l0 = lc * LC
                                if cic is None:
                                    rhs = srcs[src][0:rc, l0 + woff : l0 + woff + LC]
                                else:
                                    rhs = srcs[src][
                                        0:rc, cic, l0 + woff : l0 + woff + LC
                                    ]
                                nc.tensor.matmul(
                                    psums[lc][:],
                                    lhsT,
                                    rhs,
                                    start=(c == 0),
                                    stop=(c == N_CHUNK - 1),
                                )
                        out_sb = opool.tile([128, L], f32, name="out_sb", tag="o")
                        for lc in range(N_LC):
                            nc.vector.tensor_scalar_add(
                                out_sb[:, lc * LC : (lc + 1) * LC],
                                psums[lc][:],
                                bias_sb[:, coc : coc + 1],
                            )
                        nc.sync.dma_start(
                            out=o_d[b, coc * 128 : (coc + 1) * 128, :],
                            in_=out_sb[:],
                        )

    nc.finalize()
    return nc


def _get_nc(reps=1, probe=()):
    key = ("nc", reps, tuple(probe))
    if key not in _NC_CACHE:
        _NC_CACHE[key] = _build_nc(reps, probe)
    return _NC_CACHE[key]


def _pack_weight_chunks(w_eff):
    """[C_out, C_in, K] -> [N_CHUNK, 128, C_out] stationary chunks."""
    wT = w_eff.transpose(2, 1, 0)  # [K, C_in, C_out]
    wc = np.zeros((N_CHUNK, 128, C_OUT), np.float32)
    wc[0] = wT[0, 0:128]
    wc[1] = wT[0, 128:256]
    wc[2] = wT[1, 0:128]
    wc[3] = wT[1, 128:256]
    wc[4] = wT[2, 0:128]
    wc[5] = wT[2, 128:256]
    wc[6, 0:44] = wT[2, 256:300]
    wc[7, 0:44] = wT[0, 256:300]
    wc[7, 44:88] = wT[1, 256:300]
    return wc


def _run(inputs, trace=False, reps=1, **trace_kwargs):
    x = np.asarray(inputs["x"], dtype=np.float32)
    weight = np.asarray(inputs["weight"], dtype=np.float32)
    reg = np.asarray(inputs["words_regularization"], dtype=np.float32)
    bias = np.asarray(inputs["bias"], dtype=np.float32)

    w_eff = weight * reg[:, None, :]  # [C_out, C_in, K]
    wc = _pack_weight_chunks(w_eff)
    b_r = np.ascontiguousarray(bias.reshape(N_COC, 128, 1))
    xp = np.pad(x, ((0, 0), (0, 0), (1, 1)))  # [B, C_in, LP]
    xs = xp.reshape(N_CORES, B_LOC, C_IN, LP)

    in_maps = [
        {"x": np.ascontiguousarray(xs[i]), "w": wc, "b": b_r}
        for i in range(N_CORES)
    ]
    nc = _get_nc(reps)
    res = run_bass_kernel_spmd(
        nc, in_maps, list(range(N_CORES)), trace=trace, **trace_kwargs
    )
    out = np.concatenate(
        [res.results[i]["out"] for i in range(N_CORES)], axis=0
    )
    return out, res


def kernel(**inputs):
    out, _ = _run(inputs, trace=False)
    return out



# revision 2
# speedup vs baseline: 1.3563x; 1.3563x over previous
"""Conv1d (B=64, C_in=300, L=2048 -> C_out=512, K=3, pad=1) on 8 trn2 cores.

Strategy: data-parallel over batch (8 batches per core). Per batch, the
conv is computed as 8 accumulating bf16 matmuls per (co_chunk, l_chunk)
PSUM tile (fp32 accumulate). The contraction (ci, k) of 900 rows is
packed into 8 stationary chunks:

  c0/c1: k=0, ci 0-127 / 128-255   -> padded x   at window offset l0
  c2/c3: k=1, ci 0-127 / 128-255   -> unpadded x at window offset l0
  c4/c5: k=2, ci 0-127 / 128-255   -> padded x   at window offset l0+2
  c6:    k=2, ci 256-299 (44 rows) -> padded x   at window offset l0+2
  c7:    k=0 ci 256-299 + k=1 ci 256-299 merged (88 rows) -> m_sb at l0

The k=1 taps read a separate unpadded-x SBUF copy so every moving-operand
window starts at an even element offset, and the merged c7 chunk reads a
small materialized tile whose first 44 partitions hold padded-x rows ci
256-299 and next 44 partitions hold unpadded-x rows ci 256-299. Host
pre-packs the matching stationary weight chunks in bf16 (halves input
HBM traffic vs fp32 and enables fast weight load). x is zero-padded to
length 2050 on the host. Weights stay stationary across the 4 l-chunks
of each accumulation pass; bias is folded in during PSUM evacuation on
the vector engine, which also casts to bf16 so the output DMA is half
size; the host upcasts to fp32.
"""

import contextlib

import ml_dtypes
import numpy as np

import concourse.bass as bass
import concourse.mybir as mybir
import concourse.tile as tile
from concourse import bacc
from concourse.bass_utils import run_bass_kernel_spmd

B, C_IN, L = 64, 300, 2048
C_OUT, K = 512, 3
N_CORES = 8
B_LOC = B // N_CORES
LP = L + 2  # host-side zero-padded length
N_COC = C_OUT // 128  # co chunks of 128 partitions
LC = 512  # l chunk = one PSUM bank of fp32
N_LC = L // LC

BF16 = ml_dtypes.bfloat16

# (rows, source, cic_or_None, window_offset) per stationary chunk;
# sources: 0 = x_sb (padded), 1 = x1_sb (unpadded), 2 = m_sb (merged tail)
CHUNKS = [
    (128, 0, 0, 0),
    (128, 0, 1, 0),
    (128, 1, 0, 0),
    (128, 1, 1, 0),
    (128, 0, 0, 2),
    (128, 0, 1, 2),
    (44, 0, 2, 2),
    (88, 2, None, 0),
]
N_CHUNK = len(CHUNKS)

_NC_CACHE = {}


def _build_nc(reps=1, probe=()):
    f32 = mybir.dt.float32
    bf16 = mybir.dt.bfloat16
    nc = bacc.Bacc(None, target_bir_lowering=False)

    x_d = nc.dram_tensor("x", [B_LOC, C_IN, LP], bf16, kind="ExternalInput")
    w_d = nc.dram_tensor("w", [N_CHUNK, 128, C_OUT], bf16, kind="ExternalInput")
    b_d = nc.dram_tensor("b", [N_COC, 128, 1], f32, kind="ExternalInput")
    o_d = nc.dram_tensor("out", [B_LOC, C_OUT, L], bf16, kind="ExternalOutput")

    with tile.TileContext(nc) as tc:
        with (
            tc.tile_pool(name="wpool", bufs=1) as wpool,
            tc.tile_pool(name="xpool", bufs=2) as xpool,
            tc.tile_pool(name="opool", bufs=3) as opool,
            tc.tile_pool(name="pspool", bufs=8, space="PSUM") as pspool,
        ):
            w_sb = wpool.tile([128, N_CHUNK, C_OUT], bf16)
            for c, (rc, _, _, _) in enumerate(CHUNKS):
                nc.sync.dma_start(out=w_sb[0:rc, c, :], in_=w_d[c, 0:rc, :])
            bias_sb = wpool.tile([128, N_COC], f32)
            for coc in range(N_COC):
                nc.sync.dma_start(out=bias_sb[:, coc : coc + 1], in_=b_d[coc])

            if reps > 1:
                # Benchmark mode: repeat the whole body inside the NEFF so
                # per-iteration HW time can be isolated from RPC/transfer
                # overhead by differencing two rep counts.
                rep_stack = contextlib.ExitStack()
                rep_stack.enter_context(
                    tc.For_i(
                        0,
                        reps,
                        1,
                        hint_engines=(
                            mybir.EngineType.PE,
                            mybir.EngineType.DVE,
                            mybir.EngineType.SP,
                        ),
                    )
                )
            else:
                rep_stack = contextlib.ExitStack()

            with rep_stack:
                for b in range(B_LOC):
                    x_sb = xpool.tile([128, 3, LP], bf16, name="x_sb", tag="x")
                    x1_sb = xpool.tile(
                        [128, 2, L], bf16, name="x1_sb", tag="x1"
                    )
                    m_sb = xpool.tile([128, LP], bf16, name="m_sb", tag="m")
                    srcs = (x_sb, x1_sb, m_sb)
                    for cic, c0 in ((0, 0), (1, 128), (2, 256)):
                        cs = min(128, C_IN - c0)
                        nc.sync.dma_start(
                            out=x_sb[0:cs, cic, :], in_=x_d[b, c0 : c0 + cs, :]
                        )
                    for cic, c0 in ((0, 0), (1, 128)):
                        nc.sync.dma_start(
                            out=x1_sb[0:128, cic, :],
                            in_=x_d[b, c0 : c0 + 128, 1 : L + 1],
                        )
                    nc.sync.dma_start(
                        out=m_sb[0:44, :], in_=x_d[b, 256:300, :]
                    )
                    nc.sync.dma_start(
                        out=m_sb[44:88, 0:L], in_=x_d[b, 256:300, 1 : L + 1]
                    )

                    for coc in range(N_COC):
                        psums = [
                            pspool.tile([128, LC], f32, name="ps", tag="ps")
                            for _ in range(N_LC)
                        ]
                        # Weight-stationary: all 4 l-chunks per chunk.
                        for c, (rc, src, cic, woff) in enumerate(CHUNKS):
                            lhsT = w_sb[0:rc, c, coc * 128 : (coc + 1) * 128]
                            for lc in range(N_LC):
                                l0 = lc * LC
                                if cic is None:
                                    rhs = srcs[src][0:rc, l0 + woff : l0 + woff + LC]
                                else:
                                    rhs = srcs[src][
                                        0:rc, cic, l0 + woff : l0 + woff + LC
                                    ]
                                nc.tensor.matmul(
                                    psums[lc][:],
                                    lhsT,
                                    rhs,
                                    start=(c == 0),
                                    stop=(c == N_CHUNK - 1),
                                )
                        out_sb = opool.tile([128, L], bf16, name="out_sb", tag="o")
                        for lc in range(N_LC):
                            nc.vector.tensor_scalar_add(
                                out_sb[:, lc * LC : (lc + 1) * LC],
                                psums[lc][:],
                                bias_sb[:, coc : coc + 1],
                            )
                        nc.sync.dma_start(
                            out=o_d[b, coc * 128 : (coc + 1) * 128, :],
                            in_=out_sb[:],
                        )

    nc.finalize()
    return nc


def _get_nc(reps=1, probe=()):
    key = ("nc", reps, tuple(probe))
    if key not in _NC_CACHE:
        _NC_CACHE[key] = _build_nc(reps, probe)
    return _NC_CACHE[key]


def _pack_weight_chunks(w_eff):
    """[C_out, C_in, K] -> [N_CHUNK, 128, C_out] stationary chunks."""
    wT = w_eff.transpose(2, 1, 0)  # [K, C_in, C_out]
    wc = np.zeros((N_CHUNK, 128, C_OUT), np.float32)
    wc[0] = wT[0, 0:128]
    wc[1] = wT[0, 128:256]
    wc[2] = wT[1, 0:128]
    wc[3] = wT[1, 128:256]
    wc[4] = wT[2, 0:128]
    wc[5] = wT[2, 128:256]
    wc[6, 0:44] = wT[2, 256:300]
    wc[7, 0:44] = wT[0, 256:300]
    wc[7, 44:88] = wT[1, 256:300]
    return wc


def _run(inputs, trace=False, reps=1, **trace_kwargs):
    x = np.asarray(inputs["x"], dtype=np.float32)
    weight = np.asarray(inputs["weight"], dtype=np.float32)
    reg = np.asarray(inputs["words_regularization"], dtype=np.float32)
    bias = np.asarray(inputs["bias"], dtype=np.float32)

    w_eff = weight * reg[:, None, :]  # [C_out, C_in, K]
    wc = _pack_weight_chunks(w_eff).astype(BF16)
    b_r = np.ascontiguousarray(bias.reshape(N_COC, 128, 1))
    xp = np.pad(x, ((0, 0), (0, 0), (1, 1))).astype(BF16)  # [B, C_in, LP]
    xs = xp.reshape(N_CORES, B_LOC, C_IN, LP)

    in_maps = [
        {"x": np.ascontiguousarray(xs[i]), "w": wc, "b": b_r}
        for i in range(N_CORES)
    ]
    nc = _get_nc(reps)
    res = run_bass_kernel_spmd(
        nc, in_maps, list(range(N_CORES)), trace=trace, **trace_kwargs
    )
    out = np.concatenate(
        [np.asarray(res.results[i]["out"]) for i in range(N_CORES)], axis=0
    ).astype(np.float32)
    return out, res


def kernel(**inputs):
    out, _ = _run(inputs, trace=False)
    return out


# revision 4
# speedup vs baseline: 1.3630x; 1.0049x over previous
"""Conv1d (B=64, C_in=300, L=2048 -> C_out=512, K=3, pad=1) on 8 trn2 cores.

Strategy: data-parallel over batch (8 batches per core). Per batch, the
conv is computed as 8 accumulating bf16 matmuls per (co_chunk, l_chunk)
PSUM tile (fp32 accumulate). The contraction (ci, k) of 900 rows is
packed into 8 stationary chunks:

  c0/c1: k=0, ci 0-127 / 128-255   -> padded x   at window offset l0
  c2/c3: k=1, ci 0-127 / 128-255   -> unpadded x at window offset l0
  c4/c5: k=2, ci 0-127 / 128-255   -> padded x   at window offset l0+2
  c6:    k=2, ci 256-299 (44 rows) -> padded x   at window offset l0+2
  c7:    k=0 ci 256-299 + k=1 ci 256-299 merged (88 rows) -> m_sb at l0

The k=1 taps read a separate unpadded-x SBUF copy so every moving-operand
window starts at an even element offset, and the merged c7 chunk reads a
small materialized tile whose first 44 partitions hold padded-x rows ci
256-299 and next 44 partitions hold unpadded-x rows ci 256-299. Host
pre-packs the matching stationary weight chunks in bf16 (halves input
HBM traffic vs fp32 and enables fast weight load). x is zero-padded to
length 2050 on the host. Weights stay stationary across the 4 l-chunks
of each accumulation pass; bias is folded in during PSUM evacuation on
the vector engine, which also casts to bf16 so the output DMA is half
size; the host upcasts to fp32.
"""

import contextlib

import ml_dtypes
import numpy as np

import concourse.bass as bass
import concourse.mybir as mybir
import concourse.tile as tile
from concourse import bacc
from concourse.bass_utils import run_bass_kernel_spmd

B, C_IN, L = 64, 300, 2048
C_OUT, K = 512, 3
N_CORES = 8
B_LOC = B // N_CORES
LP = L + 2  # host-side zero-padded length
N_COC = C_OUT // 128  # co chunks of 128 partitions
LC = 512  # l chunk = one PSUM bank of fp32
N_LC = L // LC

BF16 = ml_dtypes.bfloat16

# (rows, source, cic_or_None, window_offset) per stationary chunk;
# sources: 0 = x_sb (padded), 1 = x1_sb (unpadded), 2 = m_sb (merged tail)
CHUNKS = [
    (128, 0, 0, 0),
    (128, 0, 1, 0),
    (128, 1, 0, 0),
    (128, 1, 1, 0),
    (128, 0, 0, 2),
    (128, 0, 1, 2),
    (44, 0, 2, 2),
    (88, 2, None, 0),
]
N_CHUNK = len(CHUNKS)

_NC_CACHE = {}


def _build_nc(reps=1, probe=()):
    f32 = mybir.dt.float32
    bf16 = mybir.dt.bfloat16
    nc = bacc.Bacc(None, target_bir_lowering=False)

    x_d = nc.dram_tensor("x", [B_LOC, C_IN, LP], bf16, kind="ExternalInput")
    w_d = nc.dram_tensor("w", [N_CHUNK, 128, C_OUT], bf16, kind="ExternalInput")
    b_d = nc.dram_tensor("b", [N_COC, 128, 1], f32, kind="ExternalInput")
    o_d = nc.dram_tensor("out", [B_LOC, C_OUT, L], bf16, kind="ExternalOutput")

    with tile.TileContext(nc) as tc:
        with (
            tc.tile_pool(name="wpool", bufs=1) as wpool,
            tc.tile_pool(name="xpool", bufs=2) as xpool,
            tc.tile_pool(name="opool", bufs=3) as opool,
            tc.tile_pool(name="pspool", bufs=8, space="PSUM") as pspool,
        ):
            w_sb = wpool.tile([128, N_CHUNK, C_OUT], bf16)
            for c, (rc, _, _, _) in enumerate(CHUNKS):
                nc.sync.dma_start(out=w_sb[0:rc, c, :], in_=w_d[c, 0:rc, :])
            bias_sb = wpool.tile([128, N_COC], f32)
            for coc in range(N_COC):
                nc.sync.dma_start(out=bias_sb[:, coc : coc + 1], in_=b_d[coc])

            # Warmup: dummy matmuls on junk SBUF keep the PE busy while the
            # first batch's DMAs land, so the HAM clock-gate reaches 8/8
            # (2.4 GHz) before real work starts and the real matmuls never
            # pay the cold 1.2 GHz rate.
            junk_sb = wpool.tile([128, 640], bf16)
            nc.vector.memset(junk_sb[:], 0.0)
            warm_ps = [
                pspool.tile([128, LC], f32, name="wps", tag="ps")
                for _ in range(N_LC)
            ]
            for i in range(30):
                nc.tensor.matmul(
                    warm_ps[i % N_LC][:],
                    junk_sb[:, 0:128],
                    junk_sb[:, 128:640],
                    start=True,
                    stop=True,
                    skip_group_check=True,
                )

            if reps > 1:
                # Benchmark mode: repeat the whole body inside the NEFF so
                # per-iteration HW time can be isolated from RPC/transfer
                # overhead by differencing two rep counts.
                rep_stack = contextlib.ExitStack()
                rep_stack.enter_context(
                    tc.For_i(
                        0,
                        reps,
                        1,
                        hint_engines=(
                            mybir.EngineType.PE,
                            mybir.EngineType.DVE,
                            mybir.EngineType.SP,
                        ),
                    )
                )
            else:
                rep_stack = contextlib.ExitStack()

            with rep_stack:
                for b in range(B_LOC):
                    x_sb = xpool.tile([128, 3, LP], bf16, name="x_sb", tag="x")
                    x1_sb = xpool.tile(
                        [128, 2, L], bf16, name="x1_sb", tag="x1"
                    )
                    m_sb = xpool.tile([128, LP], bf16, name="m_sb", tag="m")
                    srcs = (x_sb, x1_sb, m_sb)
                    for cic, c0 in ((0, 0), (1, 128), (2, 256)):
                        cs = min(128, C_IN - c0)
                        nc.sync.dma_start(
                            out=x_sb[0:cs, cic, :], in_=x_d[b, c0 : c0 + cs, :]
                        )
                    for cic, c0 in ((0, 0), (1, 128)):
                        nc.sync.dma_start(
                            out=x1_sb[0:128, cic, :],
                            in_=x_d[b, c0 : c0 + 128, 1 : L + 1],
                        )
                    nc.sync.dma_start(
                        out=m_sb[0:44, :], in_=x_d[b, 256:300, :]
                    )
                    nc.sync.dma_start(
                        out=m_sb[44:88, 0:L], in_=x_d[b, 256:300, 1 : L + 1]
                    )

                    for coc in range(N_COC):
                        psums = [
                            pspool.tile([128, LC], f32, name="ps", tag="ps")
                            for _ in range(N_LC)
                        ]
                        # Weight-stationary: all 4 l-chunks per chunk.
                        for c, (rc, src, cic, woff) in enumerate(CHUNKS):
                            lhsT = w_sb[0:rc, c, coc * 128 : (coc + 1) * 128]
                            for lc in range(N_LC):
                                l0 = lc * LC
                                if cic is None:
                                    rhs = srcs[src][0:rc, l0 + woff : l0 + woff + LC]
                                else:
                                    rhs = srcs[src][
                                        0:rc, cic, l0 + woff : l0 + woff + LC
                                    ]
                                nc.tensor.matmul(
                                    psums[lc][:],
                                    lhsT,
                                    rhs,
                                    start=(c == 0),
                                    stop=(c == N_CHUNK - 1),
                                )
                        out_sb = opool.tile([128, L], bf16, name="out_sb", tag="o")
                        # Evacuation split across DVE (lc 0-1) and the
                        # Activation engine (lc 2-3) so neither engine gates
                        # the tail; per-half out DMA starts as soon as its
                        # half is drained.
                        for lc in range(N_LC):
                            dst = out_sb[:, lc * LC : (lc + 1) * LC]
                            if lc < 2:
                                nc.vector.tensor_scalar_add(
                                    dst, psums[lc][:], bias_sb[:, coc : coc + 1]
                                )
                            else:
                                nc.scalar.add(
                                    dst, psums[lc][:], bias_sb[:, coc : coc + 1]
                                )
                        half = L // 2
                        for h in range(2):
                            nc.sync.dma_start(
                                out=o_d[
                                    b,
                                    coc * 128 : (coc + 1) * 128,
                                    h * half : (h + 1) * half,
                                ],
                                in_=out_sb[:, h * half : (h + 1) * half],
                            )

    nc.finalize()
    return nc


def _get_nc(reps=1, probe=()):
    key = ("nc", reps, tuple(probe))
    if key not in _NC_CACHE:
        _NC_CACHE[key] = _build_nc(reps, probe)
    return _NC_CACHE[key]


def _pack_weight_chunks(w_eff):
    """[C_out, C_in, K] -> [N_CHUNK, 128, C_out] stationary chunks."""
    wT = w_eff.transpose(2, 1, 0)  # [K, C_in, C_out]
    wc = np.zeros((N_CHUNK, 128, C_OUT), np.float32)
    wc[0] = wT[0, 0:128]
    wc[1] = wT[0, 128:256]
    wc[2] = wT[1, 0:128]
    wc[3] = wT[1, 128:256]
    wc[4] = wT[2, 0:128]
    wc[5] = wT[2, 128:256]
    wc[6, 0:44] = wT[2, 256:300]
    wc[7, 0:44] = wT[0, 256:300]
    wc[7, 44:88] = wT[1, 256:300]
    return wc


def _run(inputs, trace=False, reps=1, **trace_kwargs):
    x = np.asarray(inputs["x"], dtype=np.float32)
    weight = np.asarray(inputs["weight"], dtype=np.float32)
    reg = np.asarray(inputs["words_regularization"], dtype=np.float32)
    bias = np.asarray(inputs["bias"], dtype=np.float32)

    w_eff = weight * reg[:, None, :]  # [C_out, C_in, K]
    wc = _pack_weight_chunks(w_eff).astype(BF16)
    b_r = np.ascontiguousarray(bias.reshape(N_COC, 128, 1))
    xp = np.pad(x, ((0, 0), (0, 0), (1, 1))).astype(BF16)  # [B, C_in, LP]
    xs = xp.reshape(N_CORES, B_LOC, C_IN, LP)

    in_maps = [
        {"x": np.ascontiguousarray(xs[i]), "w": wc, "b": b_r}
        for i in range(N_CORES)
    ]
    nc = _get_nc(reps)
    res = run_bass_kernel_spmd(
        nc, in_maps, list(range(N_CORES)), trace=trace, **trace_kwargs
    )
    out = np.concatenate(
        [np.asarray(res.results[i]["out"]) for i in range(N_CORES)], axis=0
    ).astype(np.float32)
    return out, res


def kernel(**inputs):
    out, _ = _run(inputs, trace=False)
    return out
